# revision 11
# baseline (speedup 1.0000x reference)
"""GCN classifier kernel for Trainium2, data-parallel over 8 NeuronCores.

Reference computation (per batch b):
    h = emb_table[sentences[b]]                      # [S, E]
    deg = adj[b].sum(-1) + 1                         # [S]
    for (W, bias):
        z = (adj[b] @ h + h) @ W.T + 2*bias          # [S, H]
        h = relu(z / deg[:, None])
    logits[b] = max_s(h) @ Wp.T + bp                 # [C]

On-device layout: h is kept transposed (hT: [feat, seq]).  Using
    zT = uT.T @ adjT + (W @ hT) + 2b x ones
with uT = hT.T @ W.T (computed as matmul(lhsT=hT, rhs=WT)), the self-loop
(+h) term and the bias fold into the same PSUM accumulation group.
adjT tiles come from TensorE bf16 transposes of cast-DMA'd adj.
deg is computed as a [1, S] row via ones.T @ adjT matmuls so the division
broadcasts along partitions in the epilogue.
"""

import sys

import numpy as np

for _p in ("/opt/trn_rl_repo",):
    if _p not in sys.path:
        sys.path.insert(0, _p)

from contextlib import ExitStack

import concourse.bass as bass
import concourse.mybir as mybir
import concourse.tile as tile
from concourse import bacc
from concourse._compat import with_exitstack
from concourse.bass_utils import run_bass_kernel_spmd
from concourse.masks import make_identity

B, S, E, H, V, C = 64, 512, 256, 128, 50000, 2
NCORES = 8
BL = B // NCORES  # batches per core

F32 = mybir.dt.float32
BF16 = mybir.dt.bfloat16
I32 = mybir.dt.int32

P = 128
S_TILES = S // P   # 4
E_TILES = E // P   # 2


@with_exitstack
def _gcn_tile_kernel(ctx: ExitStack, tc: tile.TileContext, aps: dict):
    nc = tc.nc
    sent = aps["sentences"]
    adj = aps["adj"]
    emb = aps["emb"]
    out = aps["out"]

    consts = ctx.enter_context(tc.tile_pool(name="consts", bufs=1))

    ident = consts.tile([P, P], BF16)
    make_identity(nc, ident[:])
    ident_f32 = consts.tile([P, P], F32)
    make_identity(nc, ident_f32[:])

    ones_col = consts.tile([P, 1], BF16)
    nc.gpsimd.memset(ones_col[:], 1.0)
    ones_row = consts.tile([1, S], BF16)
    nc.gpsimd.memset(ones_row[:], 1.0)
    ones8 = consts.tile([1, BL], BF16)
    nc.gpsimd.memset(ones8[:], 1.0)

    # Weights, cast to bf16 during the load DMA (SWDGE).
    w1t = consts.tile([P, E_TILES * P], BF16)  # W1.T as 2 k-tiles [128d, 128j]
    for k in range(E_TILES):
        nc.gpsimd.dma_start(out=w1t[:, k * P:(k + 1) * P],
                            in_=aps["w1t"][k * P:(k + 1) * P, :])
    w2t = consts.tile([P, P], BF16)
    nc.gpsimd.dma_start(out=w2t[:], in_=aps["w2t"][:])
    w3t = consts.tile([P, P], BF16)
    nc.gpsimd.dma_start(out=w3t[:], in_=aps["w3t"][:])
    wpt = consts.tile([P, C], BF16)
    nc.gpsimd.dma_start(out=wpt[:], in_=aps["wpt"][:])
    bias_rows = []  # [1, H] tiles: 2*b1, 2*b2, 2*b3
    for li in range(3):
        br = consts.tile([1, H], BF16, tag=f"bias{li}")
        nc.gpsimd.dma_start(out=br[:], in_=aps["bias2"][li:li + 1, :])
        bias_rows.append(br)
    bpr = consts.tile([1, C], BF16)
    nc.gpsimd.dma_start(out=bpr[:], in_=aps["bpr"][:])

    pooledT = consts.tile([P, BL], BF16)  # max-pooled features, one col/batch

    adj_nat_p = ctx.enter_context(tc.tile_pool(name="adj_nat", bufs=2))
    adjT_p = ctx.enter_context(tc.tile_pool(name="adjT", bufs=2))
    h0_p = ctx.enter_context(tc.tile_pool(name="h0", bufs=2))
    hT_p = ctx.enter_context(tc.tile_pool(name="hT", bufs=2))
    uT_p = ctx.enter_context(tc.tile_pool(name="uT", bufs=2))
    tmp_p = ctx.enter_context(tc.tile_pool(name="tmp", bufs=2))
    idx_p = ctx.enter_context(tc.tile_pool(name="idx", bufs=2))
    r_p = ctx.enter_context(tc.tile_pool(name="r", bufs=2))

    ps_tr = ctx.enter_context(tc.tile_pool(name="ps_tr", bufs=2, space="PSUM"))
    ps_u = ctx.enter_context(tc.tile_pool(name="ps_u", bufs=2, space="PSUM"))
    ps_z = ctx.enter_context(tc.tile_pool(name="ps_z", bufs=2, space="PSUM"))
    ps_deg = ctx.enter_context(tc.tile_pool(name="ps_deg", bufs=1, space="PSUM"))

    def layer(hT_tiles, w_tiles, bias_row, adjT, r_row, out_dtype):
        """hT_tiles: list of [128, S] sbuf aps (feat x seq, bf16).
        w_tiles: list of [128, 128] W.T k-tiles.  Returns hT_next tile."""
        kt = len(hT_tiles)
        # uT[t, j] = sum_d hT[d, t] * WT[d, j]  (4 t-blocks side by side)
        psu = ps_u.tile([P, S], F32, tag="ps_u")
        for tt in range(S_TILES):
            for k in range(kt):
                nc.tensor.matmul(
                    out=psu[:, tt * P:(tt + 1) * P],
                    lhsT=hT_tiles[k][:, tt * P:(tt + 1) * P],
                    rhs=w_tiles[k][:],
                    start=(k == 0), stop=(k == kt - 1),
                )
        uT = uT_p.tile([P, S], BF16, tag="uT")
        nc.vector.tensor_copy(uT[:], psu[:])

        # zT[j, s] = sum_t uT[t, j] adjT[t, s] + sum_d WT[d, j] hT[d, s]
        #            + 2b[j] * ones[s]
        psz = ps_z.tile([P, S], F32, tag="ps_z")
        n_mm = S_TILES + kt + 1
        i = 0
        for tt in range(S_TILES):
            nc.tensor.matmul(
                out=psz[:],
                lhsT=uT[:, tt * P:(tt + 1) * P],
                rhs=adjT[:, tt * S:(tt + 1) * S],
                start=(i == 0), stop=(i == n_mm - 1),
            )
            i += 1
        for k in range(kt):
            nc.tensor.matmul(
                out=psz[:], lhsT=w_tiles[k][:], rhs=hT_tiles[k][:],
                start=(i == 0), stop=(i == n_mm - 1),
            )
            i += 1
        nc.tensor.matmul(
            out=psz[:], lhsT=bias_row, rhs=ones_row[:],
            start=False, stop=True,
        )

        # relu(z) * (1/deg)  (relu and the positive scale commute)
        tmp = tmp_p.tile([P, S], F32, tag="tmp")
        nc.scalar.activation(tmp[:], psz[:], mybir.ActivationFunctionType.Relu)
        hT_next = hT_p.tile([P, S], out_dtype, tag="hT")
        nc.vector.tensor_tensor(
            out=hT_next[:], in0=tmp[:],
            in1=r_row, op=mybir.AluOpType.mult,
        )
        return hT_next

    for b in range(BL):
        # ---- adjacency: cast-load + TensorE transpose -> adjT ----
        adj_nat = adj_nat_p.tile([P, S_TILES * S], BF16, tag="adj_nat")
        nc.gpsimd.dma_start(
            out=adj_nat[:].rearrange("p (g t) -> p g t", g=S_TILES),
            in_=adj[b].rearrange("(g p) t -> p g t", p=P),
        )
        adjT = adjT_p.tile([P, S_TILES * S], BF16, tag="adjT")
        for g in range(S_TILES):      # g: t-block (adjT partition blocks)
            pst = ps_tr.tile([P, S], BF16, tag="ps_tr")
            for i in range(S_TILES):  # i: s-block
                nc.tensor.transpose(
                    out=pst[:, i * P:(i + 1) * P],
                    in_=adj_nat[:, i * S + g * P: i * S + (g + 1) * P],
                    identity=ident[:],
                )
            nc.vector.tensor_copy(adjT[:, g * S:(g + 1) * S], pst[:])

        # ---- degree row: deg[s] = 1 + sum_t adjT[t, s] ----
        psd = ps_deg.tile([1, S], F32, tag="ps_deg")
        for g in range(S_TILES):
            nc.tensor.matmul(
                out=psd[:], lhsT=ones_col[:], rhs=adjT[:, g * S:(g + 1) * S],
                start=(g == 0), stop=(g == S_TILES - 1),
            )
        deg_row = r_p.tile([1, S], F32, tag="deg")
        nc.scalar.activation(deg_row[:], psd[:],
                             mybir.ActivationFunctionType.Copy, bias=1.0)
        r_row = r_p.tile([1, S], F32, tag="r")
        nc.vector.reciprocal(r_row[:], deg_row[:])
        # broadcast 1/deg to all partitions via outer product ones x r
        r_bf = r_p.tile([1, S], BF16, tag="rbf")
        nc.vector.tensor_copy(r_bf[:], r_row[:])
        ps_rb = ps_u.tile([P, S], F32, tag="ps_u")
        nc.tensor.matmul(out=ps_rb[:], lhsT=ones_row[:, :P], rhs=r_bf[:],
                         start=True, stop=True)
        r_bc = r_p.tile([P, S], BF16, tag="rbc")
        nc.vector.tensor_copy(r_bc[:], ps_rb[:])

        # ---- embedding gather + transpose -> h0T (bf16) ----
        idx = idx_p.tile([P, S_TILES], I32, tag="idx")
        for g in range(S_TILES):
            nc.sync.dma_start(out=idx[:, g:g + 1],
                              in_=sent[b, g * P:(g + 1) * P, None])
        h0 = h0_p.tile([P, S_TILES * E], F32, tag="h0")
        for g in range(S_TILES):
            nc.gpsimd.indirect_dma_start(
                out=h0[:, g * E:(g + 1) * E],
                out_offset=None,
                in_=emb[:],
                in_offset=bass.IndirectOffsetOnAxis(ap=idx[:, g:g + 1], axis=0),
            )
        h0T = hT_p.tile([P, E_TILES * S], BF16, tag="h0T")
        for dd in range(E_TILES):
            pst = ps_tr.tile([P, S], F32, tag="ps_tr")
            for g in range(S_TILES):
                nc.tensor.transpose(
                    out=pst[:, g * P:(g + 1) * P],
                    in_=h0[:, g * E + dd * P: g * E + (dd + 1) * P],
                    identity=ident_f32[:],
                )
            nc.vector.tensor_copy(h0T[:, dd * S:(dd + 1) * S], pst[:])

        # ---- 3 GCN layers ----
        h1 = layer([h0T[:, :S], h0T[:, S:]],
                   [w1t[:, :P], w1t[:, P:]],
                   bias_rows[0][:], adjT, r_bc[:], BF16)
        h2 = layer([h1[:]], [w2t[:]], bias_rows[1][:], adjT, r_bc[:], BF16)
        h3 = layer([h2[:]], [w3t[:]], bias_rows[2][:], adjT, r_bc[:], BF16)

        # ---- max-pool over sequence ----
        nc.vector.reduce_max(pooledT[:, b:b + 1], h3[:],
                             axis=mybir.AxisListType.X)

    # ---- classifier: logits = pooled @ Wp.T + bp ----
    psl = ps_z.tile([BL, C], F32, tag="ps_z")
    nc.tensor.matmul(out=psl[:], lhsT=pooledT[:], rhs=wpt[:],
                     start=True, stop=False)
    nc.tensor.matmul(out=psl[:], lhsT=ones8[:], rhs=bpr[:],
                     start=False, stop=True)
    logits = consts.tile([BL, C], F32)
    nc.vector.tensor_copy(logits[:], psl[:])
    nc.sync.dma_start(out=out[:], in_=logits[:])


def build_bass():
    nc = bacc.Bacc("TRN2", target_bir_lowering=False, debug=False)
    aps = {
        "sentences": nc.dram_tensor("sentences", [BL, S], I32,
                                    kind="ExternalInput").ap(),
        "adj": nc.dram_tensor("adj", [BL, S, S], F32,
                              kind="ExternalInput").ap(),
        "emb": nc.dram_tensor("emb", [V, E], F32, kind="ExternalInput").ap(),
        "w1t": nc.dram_tensor("w1t", [E, H], F32, kind="ExternalInput").ap(),
        "w2t": nc.dram_tensor("w2t", [H, H], F32, kind="ExternalInput").ap(),
        "w3t": nc.dram_tensor("w3t", [H, H], F32, kind="ExternalInput").ap(),
        "wpt": nc.dram_tensor("wpt", [H, C], F32, kind="ExternalInput").ap(),
        "bias2": nc.dram_tensor("bias2", [3, H], F32,
                                kind="ExternalInput").ap(),
        "bpr": nc.dram_tensor("bpr", [1, C], F32, kind="ExternalInput").ap(),
        "out": nc.dram_tensor("out", [BL, C], F32,
                              kind="ExternalOutput").ap(),
    }
    with tile.TileContext(nc) as tc:
        _gcn_tile_kernel(tc, aps)
    nc.compile()
    return nc


_NC_CACHE = None


def _get_nc():
    global _NC_CACHE
    if _NC_CACHE is None:
        _NC_CACHE = build_bass()
    return _NC_CACHE


def make_in_maps(sentences, adj, emb_table, W1, b1, W2, b2, W3, b3, Wp, bp):
    sentences = np.ascontiguousarray(np.asarray(sentences).astype(np.int32))
    adj = np.ascontiguousarray(np.asarray(adj, dtype=np.float32))
    emb_table = np.ascontiguousarray(np.asarray(emb_table, dtype=np.float32))
    w1t = np.ascontiguousarray(np.asarray(W1, dtype=np.float32).T)
    w2t = np.ascontiguousarray(np.asarray(W2, dtype=np.float32).T)
    w3t = np.ascontiguousarray(np.asarray(W3, dtype=np.float32).T)
    wpt = np.ascontiguousarray(np.asarray(Wp, dtype=np.float32).T)
    bias2 = np.ascontiguousarray(
        np.stack([2.0 * np.asarray(b1, dtype=np.float32),
                  2.0 * np.asarray(b2, dtype=np.float32),
                  2.0 * np.asarray(b3, dtype=np.float32)]))
    bpr = np.ascontiguousarray(np.asarray(bp, dtype=np.float32)[None, :])

    in_maps = []
    for c in range(NCORES):
        sl = slice(c * BL, (c + 1) * BL)
        in_maps.append({
            "sentences": np.ascontiguousarray(sentences[sl]),
            "adj": np.ascontiguousarray(adj[sl]),
            "emb": emb_table,
            "w1t": w1t, "w2t": w2t, "w3t": w3t, "wpt": wpt,
            "bias2": bias2, "bpr": bpr,
        })
    return in_maps


def run(in_maps, trace=False, **kw):
    nc = _get_nc()
    return run_bass_kernel_spmd(nc, in_maps, list(range(NCORES)),
                                trace=trace, **kw)


def kernel(sentences, adj, emb_table, W1, b1, W2, b2, W3, b3, Wp, bp):
    in_maps = make_in_maps(sentences, adj, emb_table,
                           W1, b1, W2, b2, W3, b3, Wp, bp)
    res = run(in_maps)
    return np.concatenate([res.results[c]["out"] for c in range(NCORES)],
                          axis=0)


# revision 12
# speedup vs baseline: 56191.6786x; 56191.6786x over previous
"""GCN classifier kernel for Trainium2, data-parallel over 8 NeuronCores.

Reference computation (per batch b):
    h = emb_table[sentences[b]]                      # [S, E]
    deg = adj[b].sum(-1) + 1                         # [S]
    for (W, bias):
        z = (adj[b] @ h + h) @ W.T + 2*bias          # [S, H]
        h = relu(z / deg[:, None])
    logits[b] = max_s(h) @ Wp.T + bp                 # [C]

On-device layout: h is kept transposed (hT: [feat, seq]).  Using
    zT = uT.T @ adjT + (W @ hT)          with uT = matmul(lhsT=hT, rhs=WT)
the self-loop (+h) term folds into the same PSUM accumulation group; the
2*bias term is the per-partition bias of the relu activation.
adj is shipped pre-transposed in bf16 (host layout prep), so adjT tiles
are a plain contiguous DMA.  deg comes out as a [1, S] row via
ones.T @ adjT matmuls so the 1/deg scale broadcasts along partitions
(materialized by a ones x r outer-product matmul).
Everything computes in bf16 with fp32 PSUM accumulation.
"""

import sys

import numpy as np

for _p in ("/opt/trn_rl_repo",):
    if _p not in sys.path:
        sys.path.insert(0, _p)

from contextlib import ExitStack

import ml_dtypes
import concourse.bass as bass
import concourse.mybir as mybir
import concourse.tile as tile
from concourse import bacc
from concourse._compat import with_exitstack
from concourse.bass_utils import run_bass_kernel_spmd
from concourse.masks import make_identity

B, S, E, H, V, C = 64, 512, 256, 128, 50000, 2
NCORES = 8
BL = B // NCORES  # batches per core

F32 = mybir.dt.float32
BF16 = mybir.dt.bfloat16
I32 = mybir.dt.int32

P = 128
S_TILES = S // P   # 4
E_TILES = E // P   # 2

BF16NP = ml_dtypes.bfloat16


@with_exitstack
def _gcn_tile_kernel(ctx: ExitStack, tc: tile.TileContext, aps: dict):
    nc = tc.nc
    sent = aps["sentences"]
    adjt = aps["adjt"]
    emb = aps["emb"]
    out = aps["out"]

    consts = ctx.enter_context(tc.tile_pool(name="consts", bufs=1))

    ident = consts.tile([P, P], BF16)
    make_identity(nc, ident[:])

    ones_col = consts.tile([P, 1], BF16)
    nc.gpsimd.memset(ones_col[:], 1.0)
    ones_row = consts.tile([1, P], BF16)
    nc.gpsimd.memset(ones_row[:], 1.0)
    ones8 = consts.tile([1, BL], BF16)
    nc.gpsimd.memset(ones8[:], 1.0)

    w1t = consts.tile([P, E_TILES * P], BF16)  # W1.T as 2 k-tiles [128d, 128j]
    nc.sync.dma_start(out=w1t[:].rearrange("p (k j) -> p k j", k=E_TILES),
                      in_=aps["w1t"].rearrange("(k p) j -> p k j", p=P))
    w2t = consts.tile([P, P], BF16)
    nc.sync.dma_start(out=w2t[:], in_=aps["w2t"][:])
    w3t = consts.tile([P, P], BF16)
    nc.sync.dma_start(out=w3t[:], in_=aps["w3t"][:])
    wpt = consts.tile([P, C], BF16)
    nc.sync.dma_start(out=wpt[:], in_=aps["wpt"][:])
    bias_col = consts.tile([P, 3], F32)  # columns: 2*b1, 2*b2, 2*b3
    nc.sync.dma_start(out=bias_col[:], in_=aps["bias2"][:])
    bpr = consts.tile([1, C], BF16)
    nc.sync.dma_start(out=bpr[:], in_=aps["bpr"][:])

    pooledT = consts.tile([P, BL], BF16)  # max-pooled features, one col/batch

    adjT_p = ctx.enter_context(tc.tile_pool(name="adjT", bufs=2))
    h0_p = ctx.enter_context(tc.tile_pool(name="h0", bufs=2))
    hT_p = ctx.enter_context(tc.tile_pool(name="hT", bufs=2))
    uT_p = ctx.enter_context(tc.tile_pool(name="uT", bufs=2))
    tmp_p = ctx.enter_context(tc.tile_pool(name="tmp", bufs=2))
    idx_p = ctx.enter_context(tc.tile_pool(name="idx", bufs=2))
    r_p = ctx.enter_context(tc.tile_pool(name="r", bufs=2))

    ps_tr = ctx.enter_context(tc.tile_pool(name="ps_tr", bufs=2, space="PSUM"))
    ps_u = ctx.enter_context(tc.tile_pool(name="ps_u", bufs=2, space="PSUM"))
    ps_z = ctx.enter_context(tc.tile_pool(name="ps_z", bufs=2, space="PSUM"))
    ps_deg = ctx.enter_context(tc.tile_pool(name="ps_deg", bufs=1, space="PSUM"))

    def layer(hT_tiles, w_tiles, bias_ap, adjT_t, r_bc, out_dtype):
        """hT_tiles: [128, S] sbuf aps (feat x seq, bf16); w_tiles: [128, 128]
        W.T k-tiles; bias_ap: [128, 1] f32; returns hT_next tile."""
        kt = len(hT_tiles)
        # uT[t, j] = sum_d hT[d, t] * WT[d, j]  (4 t-blocks side by side)
        psu = ps_u.tile([P, S], F32, tag="ps_u")
        for tt in range(S_TILES):
            for k in range(kt):
                nc.tensor.matmul(
                    out=psu[:, tt * P:(tt + 1) * P],
                    lhsT=hT_tiles[k][:, tt * P:(tt + 1) * P],
                    rhs=w_tiles[k][:],
                    start=(k == 0), stop=(k == kt - 1),
                )
        uT = uT_p.tile([P, S], BF16, tag="uT")
        nc.scalar.copy(uT[:], psu[:])

        # zT[j, s] = sum_t uT[t, j] adjT[t, s] + sum_d WT[d, j] hT[d, s]
        psz = ps_z.tile([P, S], F32, tag="ps_z")
        n_mm = S_TILES + kt
        i = 0
        for tt in range(S_TILES):
            nc.tensor.matmul(
                out=psz[:],
                lhsT=uT[:, tt * P:(tt + 1) * P],
                rhs=adjT_t[:, tt * S:(tt + 1) * S],
                start=(i == 0), stop=(i == n_mm - 1),
            )
            i += 1
        for k in range(kt):
            nc.tensor.matmul(
                out=psz[:], lhsT=w_tiles[k][:], rhs=hT_tiles[k][:],
                start=(i == 0), stop=(i == n_mm - 1),
            )
            i += 1

        # relu(z + 2b) * (1/deg)  (relu and the positive scale commute)
        tmp = tmp_p.tile([P, S], F32, tag="tmp")
        nc.scalar.activation(tmp[:], psz[:], mybir.ActivationFunctionType.Relu,
                             bias=bias_ap)
        hT_next = hT_p.tile([P, S], out_dtype, tag="hT")
        nc.vector.tensor_tensor(out=hT_next[:], in0=tmp[:], in1=r_bc,
                                op=mybir.AluOpType.mult)
        return hT_next

    for b in range(BL):
        # ---- adjacency (shipped pre-transposed, bf16): plain load ----
        adjT = adjT_p.tile([P, S_TILES * S], BF16, tag="adjT")
        nc.sync.dma_start(
            out=adjT[:].rearrange("p (g s) -> p g s", g=S_TILES),
            in_=adjt[b].rearrange("(g p) s -> p g s", p=P),
        )

        # ---- degree row: deg[s] = 1 + sum_t adjT[t, s] ----
        psd = ps_deg.tile([1, S], F32, tag="ps_deg")
        for g in range(S_TILES):
            nc.tensor.matmul(
                out=psd[:], lhsT=ones_col[:], rhs=adjT[:, g * S:(g + 1) * S],
                start=(g == 0), stop=(g == S_TILES - 1),
            )
        deg_row = r_p.tile([1, S], F32, tag="deg")
        nc.scalar.activation(deg_row[:], psd[:],
                             mybir.ActivationFunctionType.Copy, bias=1.0)
        r_row = r_p.tile([1, S], F32, tag="r")
        nc.vector.reciprocal(r_row[:], deg_row[:])
        # broadcast 1/deg to all partitions via outer product ones x r
        r_bf = r_p.tile([1, S], BF16, tag="rbf")
        nc.vector.tensor_copy(r_bf[:], r_row[:])
        ps_rb = ps_u.tile([P, S], F32, tag="ps_u")
        nc.tensor.matmul(out=ps_rb[:], lhsT=ones_row[:], rhs=r_bf[:],
                         start=True, stop=True)
        r_bc = r_p.tile([P, S], BF16, tag="rbc")
        nc.vector.tensor_copy(r_bc[:], ps_rb[:])

        # ---- embedding gather (bf16 table) + transpose -> h0T ----
        idx = idx_p.tile([P, S_TILES], I32, tag="idx")
        for g in range(S_TILES):
            nc.sync.dma_start(out=idx[:, g:g + 1],
                              in_=sent[b, g * P:(g + 1) * P, None])
        h0 = h0_p.tile([P, S_TILES * E], BF16, tag="h0")
        for g in range(S_TILES):
            nc.gpsimd.indirect_dma_start(
                out=h0[:, g * E:(g + 1) * E],
                out_offset=None,
                in_=emb[:],
                in_offset=bass.IndirectOffsetOnAxis(ap=idx[:, g:g + 1], axis=0),
            )
        h0T = hT_p.tile([P, E_TILES * S], BF16, tag="h0T")
        for dd in range(E_TILES):
            pst = ps_tr.tile([P, S], BF16, tag="ps_tr")
            for g in range(S_TILES):
                nc.tensor.transpose(
                    out=pst[:, g * P:(g + 1) * P],
                    in_=h0[:, g * E + dd * P: g * E + (dd + 1) * P],
                    identity=ident[:],
                )
            nc.vector.tensor_copy(h0T[:, dd * S:(dd + 1) * S], pst[:])

        # ---- 3 GCN layers ----
        h1 = layer([h0T[:, :S], h0T[:, S:]],
                   [w1t[:, :P], w1t[:, P:]],
                   bias_col[:, 0:1], adjT[:], r_bc[:], BF16)
        h2 = layer([h1[:]], [w2t[:]], bias_col[:, 1:2], adjT[:], r_bc[:], BF16)
        h3 = layer([h2[:]], [w3t[:]], bias_col[:, 2:3], adjT[:], r_bc[:], BF16)

        # ---- max-pool over sequence ----
        nc.vector.reduce_max(pooledT[:, b:b + 1], h3[:],
                             axis=mybir.AxisListType.X)

    # ---- classifier: logits = pooled @ Wp.T + bp ----
    psl = ps_z.tile([BL, C], F32, tag="ps_z")
    nc.tensor.matmul(out=psl[:], lhsT=pooledT[:], rhs=wpt[:],
                     start=True, stop=False)
    nc.tensor.matmul(out=psl[:], lhsT=ones8[:], rhs=bpr[:],
                     start=False, stop=True)
    logits = consts.tile([BL, C], F32)
    nc.vector.tensor_copy(logits[:], psl[:])
    nc.sync.dma_start(out=out[:], in_=logits[:])


def build_bass():
    nc = bacc.Bacc("TRN2", target_bir_lowering=False, debug=False)
    aps = {
        "sentences": nc.dram_tensor("sentences", [BL, S], I32,
                                    kind="ExternalInput").ap(),
        "adjt": nc.dram_tensor("adjt", [BL, S, S], BF16,
                               kind="ExternalInput").ap(),
        "emb": nc.dram_tensor("emb", [V, E], BF16, kind="ExternalInput").ap(),
        "w1t": nc.dram_tensor("w1t", [E, H], BF16, kind="ExternalInput").ap(),
        "w2t": nc.dram_tensor("w2t", [H, H], BF16, kind="ExternalInput").ap(),
        "w3t": nc.dram_tensor("w3t", [H, H], BF16, kind="ExternalInput").ap(),
        "wpt": nc.dram_tensor("wpt", [H, C], BF16, kind="ExternalInput").ap(),
        "bias2": nc.dram_tensor("bias2", [H, 3], F32,
                                kind="ExternalInput").ap(),
        "bpr": nc.dram_tensor("bpr", [1, C], BF16, kind="ExternalInput").ap(),
        "out": nc.dram_tensor("out", [BL, C], F32,
                              kind="ExternalOutput").ap(),
    }
    with tile.TileContext(nc) as tc:
        _gcn_tile_kernel(tc, aps)
    nc.compile()
    return nc


_NC_CACHE = None


def _get_nc():
    global _NC_CACHE
    if _NC_CACHE is None:
        _NC_CACHE = build_bass()
    return _NC_CACHE


def make_in_maps(sentences, adj, emb_table, W1, b1, W2, b2, W3, b3, Wp, bp):
    sentences = np.ascontiguousarray(np.asarray(sentences).astype(np.int32))
    adjt = np.ascontiguousarray(
        np.asarray(adj, dtype=np.float32).transpose(0, 2, 1).astype(BF16NP))
    emb_bf = np.ascontiguousarray(np.asarray(emb_table,
                                             dtype=np.float32).astype(BF16NP))
    w1t = np.ascontiguousarray(np.asarray(W1, dtype=np.float32).T.astype(BF16NP))
    w2t = np.ascontiguousarray(np.asarray(W2, dtype=np.float32).T.astype(BF16NP))
    w3t = np.ascontiguousarray(np.asarray(W3, dtype=np.float32).T.astype(BF16NP))
    wpt = np.ascontiguousarray(np.asarray(Wp, dtype=np.float32).T.astype(BF16NP))
    bias2 = np.ascontiguousarray(
        np.stack([2.0 * np.asarray(b1, dtype=np.float32),
                  2.0 * np.asarray(b2, dtype=np.float32),
                  2.0 * np.asarray(b3, dtype=np.float32)], axis=1))
    bpr = np.ascontiguousarray(
        np.asarray(bp, dtype=np.float32)[None, :].astype(BF16NP))

    in_maps = []
    for c in range(NCORES):
        sl = slice(c * BL, (c + 1) * BL)
        in_maps.append({
            "sentences": np.ascontiguousarray(sentences[sl]),
            "adjt": np.ascontiguousarray(adjt[sl]),
            "emb": emb_bf,
            "w1t": w1t, "w2t": w2t, "w3t": w3t, "wpt": wpt,
            "bias2": bias2, "bpr": bpr,
        })
    return in_maps


def run(in_maps, trace=False, **kw):
    nc = _get_nc()
    return run_bass_kernel_spmd(nc, in_maps, list(range(NCORES)),
                                trace=trace, **kw)


def kernel(sentences, adj, emb_table, W1, b1, W2, b2, W3, b3, Wp, bp):
    in_maps = make_in_maps(sentences, adj, emb_table,
                           W1, b1, W2, b2, W3, b3, Wp, bp)
    res = run(in_maps)
    return np.concatenate([res.results[c]["out"] for c in range(NCORES)],
                          axis=0)


# revision 17
# speedup vs baseline: 69564.9720x; 1.2380x over previous
"""GCN classifier kernel for Trainium2, data-parallel over 8 NeuronCores.

Reference computation (per batch b):
    h = emb_table[sentences[b]]                      # [S, E]
    deg = adj[b].sum(-1) + 1                         # [S]
    for (W, bias):
        z = (adj[b] @ h + h) @ W.T + 2*bias          # [S, H]
        h = relu(z / deg[:, None])
    logits[b] = max_s(h) @ Wp.T + bp                 # [C]

On-device layout: h is kept transposed (hT: [feat, seq]).  With
A = adj + I (self-loop folded into the adjacency, done on host where
adj is also pre-transposed and cast to bf16):
    uT = matmul(lhsT=hT, rhs=WT)         # uT[t,j] = (h @ W.T)[t,j]
    zT = uT.T @ A.T                      # = (W @ hT) @ A.T = ((A h) W.T).T
so one accumulation group of 4 matmuls per layer does both the message
passing and the self-loop term.  The 2*bias is the per-partition bias of
the relu activation.  deg+1 falls out of ones.T @ A.T matmuls as a
[1, S] row, broadcast to all partitions by a ones x deg outer-product
matmul, inverted by one [128, S] DVE reciprocal.
Everything computes in bf16 with fp32 PSUM accumulation.
"""

import sys

import numpy as np

for _p in ("/opt/trn_rl_repo",):
    if _p not in sys.path:
        sys.path.insert(0, _p)

from contextlib import ExitStack

import ml_dtypes
import concourse.bass as bass
import concourse.mybir as mybir
import concourse.tile as tile
from concourse import bacc
from concourse._compat import with_exitstack
from concourse.bass_utils import run_bass_kernel_spmd
from concourse.masks import make_identity

B, S, E, H, V, C = 64, 512, 256, 128, 50000, 2
NCORES = 8
BL = B // NCORES  # batches per core

F32 = mybir.dt.float32
BF16 = mybir.dt.bfloat16
I32 = mybir.dt.int32

P = 128
S_TILES = S // P   # 4
E_TILES = E // P   # 2

BF16NP = ml_dtypes.bfloat16


@with_exitstack
def _gcn_tile_kernel(ctx: ExitStack, tc: tile.TileContext, aps: dict):
    nc = tc.nc
    sent = aps["sentences"]
    adjt = aps["adjt"]
    emb = aps["emb"]
    out = aps["out"]

    consts = ctx.enter_context(tc.tile_pool(name="consts", bufs=1))

    ident = consts.tile([P, P], BF16)
    make_identity(nc, ident[:])

    ones_col = consts.tile([P, 1], BF16)
    nc.gpsimd.memset(ones_col[:], 1.0)
    ones_row = consts.tile([1, P], BF16)
    nc.gpsimd.memset(ones_row[:], 1.0)
    ones8 = consts.tile([1, BL], BF16)
    nc.gpsimd.memset(ones8[:], 1.0)

    w1t = consts.tile([P, E_TILES * P], BF16)  # W1.T as 2 k-tiles [128d, 128j]
    nc.sync.dma_start(out=w1t[:].rearrange("p (k j) -> p k j", k=E_TILES),
                      in_=aps["w1t"].rearrange("(k p) j -> p k j", p=P))
    w2t = consts.tile([P, P], BF16)
    nc.sync.dma_start(out=w2t[:], in_=aps["w2t"][:])
    w3t = consts.tile([P, P], BF16)
    nc.sync.dma_start(out=w3t[:], in_=aps["w3t"][:])
    wpt = consts.tile([P, C], BF16)
    nc.sync.dma_start(out=wpt[:], in_=aps["wpt"][:])
    bias_col = consts.tile([P, 3], F32)  # columns: 2*b1, 2*b2, 2*b3
    nc.sync.dma_start(out=bias_col[:], in_=aps["bias2"][:])
    bpr = consts.tile([1, C], BF16)
    nc.sync.dma_start(out=bpr[:], in_=aps["bpr"][:])

    pooledT = consts.tile([P, BL], BF16)  # max-pooled features, one col/batch

    adjT_p = ctx.enter_context(tc.tile_pool(name="adjT", bufs=2))
    h0_p = ctx.enter_context(tc.tile_pool(name="h0", bufs=2))
    hT_p = ctx.enter_context(tc.tile_pool(name="hT", bufs=2))
    uT_p = ctx.enter_context(tc.tile_pool(name="uT", bufs=2))
    tmp_p = ctx.enter_context(tc.tile_pool(name="tmp", bufs=2))
    idx_p = ctx.enter_context(tc.tile_pool(name="idx", bufs=2))
    r_p = ctx.enter_context(tc.tile_pool(name="r", bufs=2))

    ps_tr = ctx.enter_context(tc.tile_pool(name="ps_tr", bufs=2, space="PSUM"))
    ps_u = ctx.enter_context(tc.tile_pool(name="ps_u", bufs=2, space="PSUM"))
    ps_z = ctx.enter_context(tc.tile_pool(name="ps_z", bufs=2, space="PSUM"))
    ps_deg = ctx.enter_context(tc.tile_pool(name="ps_deg", bufs=1, space="PSUM"))

    def layer(hT_tiles, w_tiles, bias_ap, adjT_t, r_bc, out_dtype):
        """hT_tiles: [128, S] sbuf aps (feat x seq, bf16); w_tiles: [128, 128]
        W.T k-tiles; bias_ap: [128, 1] f32; returns hT_next tile."""
        kt = len(hT_tiles)
        # uT[t, j] = sum_d hT[d, t] * WT[d, j]  (4 t-blocks side by side)
        psu = ps_u.tile([P, S], F32, tag="ps_u")
        for tt in range(S_TILES):
            for k in range(kt):
                nc.tensor.matmul(
                    out=psu[:, tt * P:(tt + 1) * P],
                    lhsT=hT_tiles[k][:, tt * P:(tt + 1) * P],
                    rhs=w_tiles[k][:],
                    start=(k == 0), stop=(k == kt - 1),
                )
        uT = uT_p.tile([P, S], BF16, tag="uT")
        nc.vector.tensor_copy(uT[:], psu[:])

        # zT[j, s] = sum_t uT[t, j] A.T[t, s]   (A.T includes the +I term)
        psz = ps_z.tile([P, S], F32, tag="ps_z")
        for tt in range(S_TILES):
            nc.tensor.matmul(
                out=psz[:],
                lhsT=uT[:, tt * P:(tt + 1) * P],
                rhs=adjT_t[:, tt * S:(tt + 1) * S],
                start=(tt == 0), stop=(tt == S_TILES - 1),
            )

        # relu(z + 2b) * (1/deg)  (relu and the positive scale commute)
        tmp = tmp_p.tile([P, S], BF16, tag="tmp")
        nc.scalar.activation(tmp[:], psz[:], mybir.ActivationFunctionType.Relu,
                             bias=bias_ap)
        hT_next = hT_p.tile([P, S], out_dtype, tag="hT")
        nc.vector.tensor_tensor(out=hT_next[:], in0=tmp[:], in1=r_bc,
                                op=mybir.AluOpType.mult)
        return hT_next

    for b in range(BL):
        # ---- adjacency (shipped pre-transposed, bf16): plain load ----
        adjT = adjT_p.tile([P, S_TILES * S], BF16, tag="adjT")
        nc.sync.dma_start(
            out=adjT[:].rearrange("p (g s) -> p g s", g=S_TILES),
            in_=adjt[b].rearrange("(g p) s -> p g s", p=P),
        )

        # ---- degree row: deg[s] = sum_t A.T[t, s]  (+1 is in A's diag) ----
        psd = ps_deg.tile([1, S], F32, tag="ps_deg")
        for g in range(S_TILES):
            nc.tensor.matmul(
                out=psd[:], lhsT=ones_col[:], rhs=adjT[:, g * S:(g + 1) * S],
                start=(g == 0), stop=(g == S_TILES - 1),
            )
        deg_bf = r_p.tile([1, S], BF16, tag="deg")
        nc.scalar.copy(deg_bf[:], psd[:])
        # broadcast deg to all partitions (ones x deg), then invert on DVE
        ps_rb = ps_u.tile([P, S], F32, tag="ps_u")
        nc.tensor.matmul(out=ps_rb[:], lhsT=ones_row[:], rhs=deg_bf[:],
                         start=True, stop=True)
        r_bc = r_p.tile([P, S], BF16, tag="rbc")
        with nc.allow_low_precision(reason="1/deg at bf16 is plenty"):
            nc.vector.reciprocal(r_bc[:], ps_rb[:])

        # ---- embedding gather (bf16 table) + transpose -> h0T ----
        idx = idx_p.tile([P, S_TILES], I32, tag="idx")
        for g in range(S_TILES):
            nc.sync.dma_start(out=idx[:, g:g + 1],
                              in_=sent[b, g * P:(g + 1) * P, None])
        h0 = h0_p.tile([P, S_TILES * E], BF16, tag="h0")
        for g in range(S_TILES):
            nc.gpsimd.indirect_dma_start(
                out=h0[:, g * E:(g + 1) * E],
                out_offset=None,
                in_=emb[:],
                in_offset=bass.IndirectOffsetOnAxis(ap=idx[:, g:g + 1], axis=0),
            )
        h0T = hT_p.tile([P, E_TILES * S], BF16, tag="h0T")
        for dd in range(E_TILES):
            pst = ps_tr.tile([P, S], BF16, tag="ps_tr")
            for g in range(S_TILES):
                nc.tensor.transpose(
                    out=pst[:, g * P:(g + 1) * P],
                    in_=h0[:, g * E + dd * P: g * E + (dd + 1) * P],
                    identity=ident[:],
                )
            nc.vector.tensor_copy(h0T[:, dd * S:(dd + 1) * S], pst[:])

        # ---- 3 GCN layers ----
        h1 = layer([h0T[:, :S], h0T[:, S:]],
                   [w1t[:, :P], w1t[:, P:]],
                   bias_col[:, 0:1], adjT[:], r_bc[:], BF16)
        h2 = layer([h1[:]], [w2t[:]], bias_col[:, 1:2], adjT[:], r_bc[:], BF16)
        h3 = layer([h2[:]], [w3t[:]], bias_col[:, 2:3], adjT[:], r_bc[:], BF16)

        # ---- max-pool over sequence ----
        nc.vector.reduce_max(pooledT[:, b:b + 1], h3[:],
                             axis=mybir.AxisListType.X)

    # ---- classifier: logits = pooled @ Wp.T + bp ----
    psl = ps_z.tile([BL, C], F32, tag="ps_z")
    nc.tensor.matmul(out=psl[:], lhsT=pooledT[:], rhs=wpt[:],
                     start=True, stop=False)
    nc.tensor.matmul(out=psl[:], lhsT=ones8[:], rhs=bpr[:],
                     start=False, stop=True)
    logits = consts.tile([BL, C], F32)
    nc.vector.tensor_copy(logits[:], psl[:])
    nc.sync.dma_start(out=out[:], in_=logits[:])


def build_bass():
    nc = bacc.Bacc("TRN2", target_bir_lowering=False, debug=False)
    aps = {
        "sentences": nc.dram_tensor("sentences", [BL, S], I32,
                                    kind="ExternalInput").ap(),
        "adjt": nc.dram_tensor("adjt", [BL, S, S], BF16,
                               kind="ExternalInput").ap(),
        "emb": nc.dram_tensor("emb", [V, E], BF16, kind="ExternalInput").ap(),
        "w1t": nc.dram_tensor("w1t", [E, H], BF16, kind="ExternalInput").ap(),
        "w2t": nc.dram_tensor("w2t", [H, H], BF16, kind="ExternalInput").ap(),
        "w3t": nc.dram_tensor("w3t", [H, H], BF16, kind="ExternalInput").ap(),
        "wpt": nc.dram_tensor("wpt", [H, C], BF16, kind="ExternalInput").ap(),
        "bias2": nc.dram_tensor("bias2", [H, 3], F32,
                                kind="ExternalInput").ap(),
        "bpr": nc.dram_tensor("bpr", [1, C], BF16, kind="ExternalInput").ap(),
        "out": nc.dram_tensor("out", [BL, C], F32,
                              kind="ExternalOutput").ap(),
    }
    with tile.TileContext(nc) as tc:
        _gcn_tile_kernel(tc, aps)
    nc.compile()
    return nc


_NC_CACHE = None


def _get_nc():
    global _NC_CACHE
    if _NC_CACHE is None:
        _NC_CACHE = build_bass()
    return _NC_CACHE


def make_in_maps(sentences, adj, emb_table, W1, b1, W2, b2, W3, b3, Wp, bp):
    sentences = np.ascontiguousarray(np.asarray(sentences).astype(np.int32))
    # A.T = adj.T + I: fold the self-loop into the adjacency
    adjt = np.asarray(adj, dtype=np.float32).transpose(0, 2, 1).copy()
    _ar = np.arange(S)
    adjt[:, _ar, _ar] += 1.0
    adjt = np.ascontiguousarray(adjt.astype(BF16NP))
    emb_bf = np.ascontiguousarray(np.asarray(emb_table,
                                             dtype=np.float32).astype(BF16NP))
    w1t = np.ascontiguousarray(np.asarray(W1, dtype=np.float32).T.astype(BF16NP))
    w2t = np.ascontiguousarray(np.asarray(W2, dtype=np.float32).T.astype(BF16NP))
    w3t = np.ascontiguousarray(np.asarray(W3, dtype=np.float32).T.astype(BF16NP))
    wpt = np.ascontiguousarray(np.asarray(Wp, dtype=np.float32).T.astype(BF16NP))
    bias2 = np.ascontiguousarray(
        np.stack([2.0 * np.asarray(b1, dtype=np.float32),
                  2.0 * np.asarray(b2, dtype=np.float32),
                  2.0 * np.asarray(b3, dtype=np.float32)], axis=1))
    bpr = np.ascontiguousarray(
        np.asarray(bp, dtype=np.float32)[None, :].astype(BF16NP))

    in_maps = []
    for c in range(NCORES):
        sl = slice(c * BL, (c + 1) * BL)
        in_maps.append({
            "sentences": np.ascontiguousarray(sentences[sl]),
            "adjt": np.ascontiguousarray(adjt[sl]),
            "emb": emb_bf,
            "w1t": w1t, "w2t": w2t, "w3t": w3t, "wpt": wpt,
            "bias2": bias2, "bpr": bpr,
        })
    return in_maps


def run(in_maps, trace=False, **kw):
    nc = _get_nc()
    return run_bass_kernel_spmd(nc, in_maps, list(range(NCORES)),
                                trace=trace, **kw)


def kernel(sentences, adj, emb_table, W1, b1, W2, b2, W3, b3, Wp, bp):
    in_maps = make_in_maps(sentences, adj, emb_table,
                           W1, b1, W2, b2, W3, b3, Wp, bp)
    res = run(in_maps)
    return np.concatenate([res.results[c]["out"] for c in range(NCORES)],
                          axis=0)
